# revision 6
# baseline (speedup 1.0000x reference)
# HGNNP hypergraph convolution on 8 Trainium2 NeuronCores (Bass/Tile).
#
# Reference computation:
#   H      = relu(X @ W.T + b)                    [N, 128]
#   e_feat = segment_mean(H[v_idx], e_idx, E)     [E, 128]
#   out    = relu(segment_mean(e_feat[e_idx], v_idx, N))
#
# Strategy: DENSE block-matmul formulation (zero gather descriptors).
#   On this part, indexed-DMA (dma_gather / dma_scatter_add) is descriptor-
#   rate-bound at ~8 ns per 256B descriptor regardless of payload or memory
#   (measured), so any per-entry gather design costs >= 2*NNZ/8 * 8ns ~ 6.4 ms
#   per core.  Instead we materialize the incidence matrix A (0/1 counts) as
#   fp8 tiles on the host and stream it from HBM at bulk rate (~242 GB/s):
#     phase A:  esum_cm[c, e]  = sum_vb  H_blk[vb]^T     @ A1[vb, e-chunk]
#     phase B:  out_cm[c, v]   = sum_eb  efeat_blk[eb]^T @ A2[eb, v-chunk]
#   Both phases contract on the TensorEngine with the small dense operand
#   (H block / e_feat block, fp16) stationary and fp8 incidence chunks
#   streaming.  fp8 e4m3 holds small integer counts exactly, so the
#   segment sums are exact; per-core partial edge sums are AllReduced in
#   fp16 and scaled by 1/deg.  ~315 MB of A per phase per core at bulk HBM
#   rate ~= 1.3 ms/phase, matching the memory roofline for this regime.
import numpy as np

P = 128

N_VERTICES = 100000
N_EDGES = 25000
IN_CH = 256
OUT_CH = 128
N_CORES = 8

VPC = 12544              # vertices per core (98 blocks of 128)
VB = VPC // P            # 98
NV_PAD = N_CORES * VPC   # 100352
NE_PAD = 25088           # 196 blocks of 128
EB = NE_PAD // P         # 196

# phase A: 49 e-chunks of 512, in 7 passes x 7 chunks (7 PSUM banks)
ECW = 512
EPASS, ECHK = 7, 7       # 7*7*512 == 25088
# phase B: 25 v-chunks of 512 on 12800 padded rows, 5 passes x 5 chunks
VCW = 512
VPASS, VCHK = 5, 5       # 5*5*512 == 12800
VPC_B = VPASS * VCHK * VCW   # 12800
VBB = VPC_B // P         # 100 output blocks

_PROG_CACHE = {}
LAST_RESULTS = None      # BassKernelResults of the most recent run (for test.py)
LAST_IN_MAPS = None      # packed per-core inputs of the most recent kernel()


def build_program():
    import concourse.mybir as mybir
    import concourse.tile as tile
    from concourse import bacc

    dt = mybir.dt
    KC = IN_CH // P      # 2

    nc = bacc.Bacc("TRN2", target_bir_lowering=False, debug=False,
                   num_devices=N_CORES)

    # ---- I/O ----
    xt = nc.dram_tensor("xt", [IN_CH, VPC], dt.float16, kind="ExternalInput")
    wt = nc.dram_tensor("wt", [IN_CH, OUT_CH], dt.float16, kind="ExternalInput")
    bmat = nc.dram_tensor("bmat", [P, OUT_CH], dt.float32, kind="ExternalInput")
    ident = nc.dram_tensor("ident", [P, P], dt.float16, kind="ExternalInput")
    a1 = nc.dram_tensor("a1", [EPASS * VB * P, ECHK * ECW], dt.float8e4,
                        kind="ExternalInput")
    a2 = nc.dram_tensor("a2", [VPASS * EB * P, VCHK * VCW], dt.float8e4,
                        kind="ExternalInput")
    re_p = nc.dram_tensor("re", [P, EB], dt.float32, kind="ExternalInput")
    rv_p = nc.dram_tensor("rv", [P, VBB], dt.float32, kind="ExternalInput")
    out = nc.dram_tensor("out", [VPC_B, OUT_CH], dt.float32,
                         kind="ExternalOutput")

    # ---- internal DRAM ----
    esum = nc.dram_tensor("esum", [P, NE_PAD], dt.float16)          # ch-major
    esum_red = nc.dram_tensor("esum_red", [P, NE_PAD], dt.float16,
                              addr_space="Shared")

    with tile.TileContext(nc) as tc:
        import contextlib
        with contextlib.ExitStack() as ctx:
            const = ctx.enter_context(tc.tile_pool(name="const", bufs=1))
            hpool = ctx.enter_context(tc.tile_pool(name="hpool", bufs=1))
            efpool = ctx.enter_context(tc.tile_pool(name="efpool", bufs=1))
            apool = ctx.enter_context(tc.tile_pool(name="apool", bufs=3))
            work = ctx.enter_context(tc.tile_pool(name="work", bufs=3))
            # 7 rotating PSUM slot names (one bank each); all stages share
            psA = ctx.enter_context(tc.tile_pool(name="psA", bufs=1,
                                                 space="PSUM"))

            # ---- constants ----
            xt_sb = const.tile([P, KC, VPC], dt.float16)
            for k in range(KC):
                nc.sync.dma_start(out=xt_sb[:, k, :], in_=xt[k * P:(k + 1) * P, :])
            wt_sb = const.tile([P, KC, OUT_CH], dt.float16)
            for k in range(KC):
                nc.sync.dma_start(out=wt_sb[:, k, :], in_=wt[k * P:(k + 1) * P, :])
            bb = const.tile([P, OUT_CH], dt.float32)
            nc.sync.dma_start(out=bb[:], in_=bmat[:, :])
            id_sb = const.tile([P, P], dt.float16)
            nc.sync.dma_start(out=id_sb[:], in_=ident[:, :])
            re_sb = const.tile([P, EB], dt.float32)
            nc.sync.dma_start(out=re_sb[:], in_=re_p[:, :])
            rv_sb = const.tile([P, VBB], dt.float32)
            nc.sync.dma_start(out=rv_sb[:], in_=rv_p[:, :])

            # ---- stage H: H = relu(X @ W.T + b), fp16 blocks in SBUF ----
            # h_sb[vr, vb, c] = H[vb*128+vr, c]
            h_sb = hpool.tile([P, VB, OUT_CH], dt.float16)
            for vb in range(VB):
                ps = psA.tile([P, OUT_CH], dt.float32, space="PSUM",
                              name=f"ps{vb % 2}")
                for k in range(KC):
                    nc.tensor.matmul(out=ps[:],
                                     lhsT=xt_sb[:, k, vb * P:(vb + 1) * P],
                                     rhs=wt_sb[:, k, :],
                                     start=(k == 0), stop=(k == KC - 1))
                tmp = work.tile([P, OUT_CH], dt.float32)
                nc.vector.tensor_add(out=tmp[:], in0=ps[:], in1=bb[:])
                nc.vector.tensor_scalar_max(out=h_sb[:, vb, :], in0=tmp[:],
                                            scalar1=0.0)

            # ---- phase A: esum_cm[c, e] = sum_vb H[vb]^T @ A1[vb, echunk] ----
            for sp in range(EPASS):
                pss = [psA.tile([P, ECW], dt.float32, space="PSUM",
                                name=f"ps{j}") for j in range(ECHK)]
                for vb in range(VB):
                    a1t = apool.tile([P, ECHK * ECW], dt.float8e4)
                    r0 = (sp * VB + vb) * P
                    nc.sync.dma_start(out=a1t[:], in_=a1[r0:r0 + P, :])
                    for j in range(ECHK):
                        nc.tensor.matmul(out=pss[j][:],
                                         lhsT=h_sb[:, vb, :],
                                         rhs=a1t[:, j * ECW:(j + 1) * ECW],
                                         start=(vb == 0), stop=(vb == VB - 1))
                for j in range(ECHK):
                    es = work.tile([P, ECW], dt.float16)
                    nc.vector.tensor_copy(out=es[:], in_=pss[j][:])
                    c0 = (sp * ECHK + j) * ECW
                    nc.sync.dma_start(out=esum[:, c0:c0 + ECW], in_=es[:])

            # ---- AllReduce partial edge sums (channel-major, fp16) ----
            nc.gpsimd.collective_compute(
                "AllReduce", mybir.AluOpType.add,
                replica_groups=[list(range(N_CORES))],
                ins=[esum.ap().opt()], outs=[esum_red.ap().opt()])

            # ---- e_feat blocks: transpose each eb block, scale by 1/e_deg ----
            # ef_sb[er, eb, c] = esum_red[c, eb*128+er] * re[er, eb]
            ef_sb = efpool.tile([P, EB, OUT_CH], dt.float16)
            EBG = 28                         # eb blocks per bulk load
            for g in range(EB // EBG):
                ech = apool.tile([P, EBG * P], dt.float16)
                nc.sync.dma_start(out=ech[:],
                                  in_=esum_red[:, g * EBG * P:(g + 1) * EBG * P])
                for s in range(EBG):
                    eb = g * EBG + s
                    pst = psA.tile([P, P], dt.float16, space="PSUM",
                                   name=f"ps{s % 2}")
                    nc.tensor.transpose(pst[:], ech[:, s * P:(s + 1) * P],
                                        id_sb[:])
                    nc.vector.tensor_scalar_mul(out=ef_sb[:, eb, :], in0=pst[:],
                                                scalar1=re_sb[:, eb:eb + 1])

            # ---- phase B: out_cm[c, v] = sum_eb ef[eb]^T @ A2[eb, vchunk] ----
            for sp in range(VPASS):
                pss = [psA.tile([P, VCW], dt.float32, space="PSUM",
                                name=f"ps{j}") for j in range(VCHK)]
                for eb in range(EB):
                    a2t = apool.tile([P, VCHK * VCW], dt.float8e4)
                    r0 = (sp * EB + eb) * P
                    nc.sync.dma_start(out=a2t[:], in_=a2[r0:r0 + P, :])
                    for j in range(VCHK):
                        nc.tensor.matmul(out=pss[j][:],
                                         lhsT=ef_sb[:, eb, :],
                                         rhs=a2t[:, j * VCW:(j + 1) * VCW],
                                         start=(eb == 0), stop=(eb == EB - 1))
                for j in range(VCHK):
                    cm = work.tile([P, VCW], dt.float16)
                    nc.vector.tensor_copy(out=cm[:], in_=pss[j][:])
                    for b in range(VCW // P):
                        vbb = (sp * VCHK + j) * (VCW // P) + b
                        pst = psA.tile([P, P], dt.float16, space="PSUM",
                                       name=f"ps{5 + b % 2}")
                        nc.tensor.transpose(pst[:], cm[:, b * P:(b + 1) * P],
                                            id_sb[:])
                        ot = work.tile([P, OUT_CH], dt.float32)
                        nc.vector.tensor_scalar(out=ot[:], in0=pst[:],
                                                scalar1=rv_sb[:, vbb:vbb + 1],
                                                scalar2=0.0,
                                                op0=mybir.AluOpType.mult,
                                                op1=mybir.AluOpType.max)
                        nc.sync.dma_start(out=out[vbb * P:(vbb + 1) * P, :],
                                          in_=ot[:])

    nc.compile()
    return nc


def pack_inputs(X, W, b, v_idx, e_idx):
    """Host-side preprocessing: build per-core fp8 incidence tiles in the
    pass/block-chunk layouts the device program streams, plus dense inputs."""
    import ml_dtypes
    f16, f32 = np.float16, np.float32
    f8 = ml_dtypes.float8_e4m3

    v = np.asarray(v_idx).astype(np.int64)
    e = np.asarray(e_idx).astype(np.int64)

    # fp8 byte LUT for small counts (0..15); counts beyond 15 are impossible
    # for random data but clip defensively (value error stays tiny/local).
    lut = np.arange(16, dtype=np.float32).astype(f8).view(np.uint8)

    # dense inputs
    xt_full = np.zeros((IN_CH, NV_PAD), f16)
    xt_full[:, :N_VERTICES] = np.asarray(X, f32).T.astype(f16)
    wt = np.ascontiguousarray(np.asarray(W, f32).T.astype(f16))
    bmat = np.tile(np.asarray(b, f32)[None, :], (P, 1))
    ident = np.eye(P, dtype=f16)

    # degree reciprocals
    edeg = np.bincount(e, minlength=NE_PAD).astype(f32)
    re = (1.0 / np.maximum(edeg, 1.0)).astype(f32)
    re_p = np.ascontiguousarray(re.reshape(EB, P).T)          # [er, eb]
    vdeg = np.bincount(v, minlength=N_CORES * VPC_B).astype(f32)
    rv = (1.0 / np.maximum(vdeg, 1.0)).astype(f32)

    core = v // VPC
    vl = v - core * VPC

    def counts_to_f8(cnt_u8, rows, cols):
        # fp8 e4m3 byte for 1.0 is 0x38; counts are overwhelmingly 0/1, so a
        # byte-multiply covers them and the rare multi-edges get LUT-fixed.
        out = cnt_u8 * np.uint8(0x38)
        fix = np.flatnonzero(cnt_u8 > 1)
        if fix.size:
            out[fix] = lut[np.minimum(cnt_u8[fix], 15)].copy()
        return out.view(f8).reshape(rows, cols)

    in_maps = []
    for c in range(N_CORES):
        m = core == c
        vc, ec = vl[m], e[m]

        # a1[(sp*VB+vb)*P + vr, ecp*ECW + el] = count(v==vb*P+vr,
        #                                             e==(sp*ECHK+ecp)*ECW+el)
        EC = ECHK * ECW
        row1 = (ec // EC * VB + vc // P) * P + vc % P
        cnt = np.zeros(EPASS * VB * P * EC, np.uint8)
        np.add.at(cnt, row1 * EC + ec % EC, 1)
        a1 = counts_to_f8(cnt, EPASS * VB * P, EC)
        del cnt

        # a2[(sp*EB+eb)*P + er, vcp*VCW + vl] = count(e==eb*P+er,
        #                                             v==(sp*VCHK+vcp)*VCW+vl)
        VC = VCHK * VCW
        row2 = (vc // VC * EB + ec // P) * P + ec % P
        cnt = np.zeros(VPASS * EB * P * VC, np.uint8)
        np.add.at(cnt, row2 * VC + vc % VC, 1)
        a2 = counts_to_f8(cnt, VPASS * EB * P, VC)
        del cnt

        rv_core = rv[c * VPC:(c + 1) * VPC]
        rv_pad = np.zeros(VPC_B, f32)
        rv_pad[:VPC] = rv_core
        in_maps.append({
            "xt": np.ascontiguousarray(xt_full[:, c * VPC:(c + 1) * VPC]),
            "wt": wt,
            "bmat": bmat,
            "ident": ident,
            "a1": a1,
            "a2": a2,
            "re": re_p,
            "rv": np.ascontiguousarray(rv_pad.reshape(VBB, P).T),
        })
    return in_maps


def run(in_maps, trace=False):
    global LAST_RESULTS
    from concourse.bass_utils import run_bass_kernel_spmd
    if "prog" not in _PROG_CACHE:
        _PROG_CACHE["prog"] = build_program()
    nc = _PROG_CACHE["prog"]
    res = run_bass_kernel_spmd(nc, in_maps, core_ids=list(range(N_CORES)),
                               trace=trace)
    LAST_RESULTS = res
    return res


def kernel(X, W, b, v_idx, e_idx, trace=False):
    global LAST_IN_MAPS
    in_maps = pack_inputs(X, W, b, v_idx, e_idx)
    LAST_IN_MAPS = in_maps
    res = run(in_maps, trace=trace)
    out = np.concatenate([res.results[c]["out"][:VPC] for c in range(N_CORES)],
                         axis=0)
    return np.ascontiguousarray(out[:N_VERTICES]).astype(np.float32)


# revision 8
# speedup vs baseline: 1.0967x; 1.0967x over previous
# HGNNP hypergraph convolution on 8 Trainium2 NeuronCores (Bass/Tile).
#
# Reference computation:
#   H      = relu(X @ W.T + b)                    [N, 128]
#   e_feat = segment_mean(H[v_idx], e_idx, E)     [E, 128]
#   out    = relu(segment_mean(e_feat[e_idx], v_idx, N))
#
# Strategy: DENSE block-matmul formulation (zero gather descriptors).
#   On this part, indexed-DMA (dma_gather / dma_scatter_add) is descriptor-
#   rate-bound at ~8 ns per 256B descriptor regardless of payload or memory
#   (measured), so any per-entry gather design costs >= 2*NNZ/8 * 8ns ~ 6.4 ms
#   per core.  Instead we materialize the incidence matrix A (0/1 counts) as
#   fp8 tiles on the host and stream it from HBM at bulk rate (~242 GB/s):
#     phase A:  esum_cm[c, e]  = sum_vb  H_blk[vb]^T     @ A1[vb, e-chunk]
#     phase B:  out_cm[c, v]   = sum_eb  efeat_blk[eb]^T @ A2[eb, v-chunk]
#   Both phases contract on the TensorEngine with the small dense operand
#   (H block / e_feat block, fp16) stationary and fp8 incidence chunks
#   streaming.  fp8 e4m3 holds small integer counts exactly, so the
#   segment sums are exact; per-core partial edge sums are AllReduced in
#   fp16 and scaled by 1/deg.  ~315 MB of A per phase per core at bulk HBM
#   rate ~= 1.3 ms/phase, matching the memory roofline for this regime.
import numpy as np

P = 128

N_VERTICES = 100000
N_EDGES = 25000
IN_CH = 256
OUT_CH = 128
N_CORES = 8

VPC = 12544              # vertices per core (98 blocks of 128)
VB = VPC // P            # 98
NV_PAD = N_CORES * VPC   # 100352
NE_PAD = 25088           # 196 blocks of 128
EB = NE_PAD // P         # 196

# phase A: 49 e-chunks of 512, in 7 passes x 7 chunks (7 PSUM banks)
ECW = 512
EPASS, ECHK = 7, 7       # 7*7*512 == 25088
# phase B: 25 v-chunks of 512 on 12800 padded rows, 5 passes x 5 chunks
VCW = 512
VPASS, VCHK = 5, 5       # 5*5*512 == 12800
VPC_B = VPASS * VCHK * VCW   # 12800
VBB = VPC_B // P         # 100 output blocks

_PROG_CACHE = {}
LAST_RESULTS = None      # BassKernelResults of the most recent run (for test.py)
LAST_IN_MAPS = None      # packed per-core inputs of the most recent kernel()


def build_program():
    import concourse.mybir as mybir
    import concourse.tile as tile
    from concourse import bacc

    dt = mybir.dt
    KC = IN_CH // P      # 2

    nc = bacc.Bacc("TRN2", target_bir_lowering=False, debug=False,
                   num_devices=N_CORES)

    # ---- I/O ----
    xt = nc.dram_tensor("xt", [IN_CH, VPC], dt.float16, kind="ExternalInput")
    wt = nc.dram_tensor("wt", [IN_CH, OUT_CH], dt.float16, kind="ExternalInput")
    bmat = nc.dram_tensor("bmat", [P, OUT_CH], dt.float32, kind="ExternalInput")
    ident = nc.dram_tensor("ident", [P, P], dt.float16, kind="ExternalInput")
    a1 = nc.dram_tensor("a1", [EPASS * VB * P, ECHK * ECW], dt.float8e4,
                        kind="ExternalInput")
    a2 = nc.dram_tensor("a2", [VPASS * EB * P, VCHK * VCW], dt.float8e4,
                        kind="ExternalInput")
    re_p = nc.dram_tensor("re", [P, EB], dt.float32, kind="ExternalInput")
    rv_p = nc.dram_tensor("rv", [P, VBB], dt.float32, kind="ExternalInput")
    out = nc.dram_tensor("out", [VPC_B, OUT_CH], dt.float32,
                         kind="ExternalOutput")

    # ---- internal DRAM ----
    # pass-major slabs so each pass's partial sums AllReduce independently,
    # overlapping the collective with later phase-A passes and phase B
    esum = nc.dram_tensor("esum", [EPASS, P, ECHK * ECW], dt.float16)
    esum_red = nc.dram_tensor("esum_red", [EPASS, P, ECHK * ECW], dt.float16,
                              addr_space="Shared")

    with tile.TileContext(nc) as tc:
        import contextlib
        with contextlib.ExitStack() as ctx:
            const = ctx.enter_context(tc.tile_pool(name="const", bufs=1))
            hpool = ctx.enter_context(tc.tile_pool(name="hpool", bufs=1))
            efpool = ctx.enter_context(tc.tile_pool(name="efpool", bufs=1))
            a1pool = ctx.enter_context(tc.tile_pool(name="a1pool", bufs=4))
            a2pool = ctx.enter_context(tc.tile_pool(name="a2pool", bufs=4))
            echpool = ctx.enter_context(tc.tile_pool(name="echpool", bufs=2))
            work = ctx.enter_context(tc.tile_pool(name="work", bufs=3))
            # 7 rotating PSUM slot names (one bank each); all stages share
            psA = ctx.enter_context(tc.tile_pool(name="psA", bufs=1,
                                                 space="PSUM"))

            # ---- constants ----
            xt_sb = const.tile([P, KC, VPC], dt.float16)
            for k in range(KC):
                nc.sync.dma_start(out=xt_sb[:, k, :], in_=xt[k * P:(k + 1) * P, :])
            wt_sb = const.tile([P, KC, OUT_CH], dt.float16)
            for k in range(KC):
                nc.sync.dma_start(out=wt_sb[:, k, :], in_=wt[k * P:(k + 1) * P, :])
            bb = const.tile([P, OUT_CH], dt.float32)
            nc.sync.dma_start(out=bb[:], in_=bmat[:, :])
            id_sb = const.tile([P, P], dt.float16)
            nc.sync.dma_start(out=id_sb[:], in_=ident[:, :])
            re_sb = const.tile([P, EB], dt.float32)
            nc.sync.dma_start(out=re_sb[:], in_=re_p[:, :])
            rv_sb = const.tile([P, VBB], dt.float32)
            nc.sync.dma_start(out=rv_sb[:], in_=rv_p[:, :])

            # ---- stage H: H = relu(X @ W.T + b), fp16 blocks in SBUF ----
            # h_sb[vr, vb, c] = H[vb*128+vr, c]
            h_sb = hpool.tile([P, VB, OUT_CH], dt.float16)
            for vb in range(VB):
                ps = psA.tile([P, OUT_CH], dt.float32, space="PSUM",
                              name=f"ps{vb % 4}")
                for k in range(KC):
                    nc.tensor.matmul(out=ps[:],
                                     lhsT=xt_sb[:, k, vb * P:(vb + 1) * P],
                                     rhs=wt_sb[:, k, :],
                                     start=(k == 0), stop=(k == KC - 1))
                tmp = work.tile([P, OUT_CH], dt.float32)
                nc.vector.tensor_add(out=tmp[:], in0=ps[:], in1=bb[:])
                nc.vector.tensor_scalar_max(out=h_sb[:, vb, :], in0=tmp[:],
                                            scalar1=0.0)

            # ---- phase A: esum_cm[c, e] = sum_vb H[vb]^T @ A1[vb, echunk] ----
            for sp in range(EPASS):
                pss = [psA.tile([P, ECW], dt.float32, space="PSUM",
                                name=f"ps{j}") for j in range(ECHK)]
                for vb in range(VB):
                    a1t = a1pool.tile([P, ECHK * ECW], dt.float8e4)
                    r0 = (sp * VB + vb) * P
                    nc.sync.dma_start(out=a1t[:], in_=a1[r0:r0 + P, :])
                    for j in range(ECHK):
                        nc.tensor.matmul(out=pss[j][:],
                                         lhsT=h_sb[:, vb, :],
                                         rhs=a1t[:, j * ECW:(j + 1) * ECW],
                                         start=(vb == 0), stop=(vb == VB - 1))
                for j in range(ECHK):
                    es = work.tile([P, ECW], dt.float16)
                    nc.vector.tensor_copy(out=es[:], in_=pss[j][:])
                    nc.sync.dma_start(out=esum[sp, :, j * ECW:(j + 1) * ECW],
                                      in_=es[:])
                # per-pass AllReduce of this slab (overlaps later passes)
                nc.gpsimd.collective_compute(
                    "AllReduce", mybir.AluOpType.add,
                    replica_groups=[list(range(N_CORES))],
                    ins=[esum[sp].opt()], outs=[esum_red[sp].opt()])

            # ---- e_feat blocks: transpose each eb block, scale by 1/e_deg ----
            # ef_sb[er, eb, c] = esum_red[c, eb*128+er] * re[er, eb]
            ef_sb = efpool.tile([P, EB, OUT_CH], dt.float16)
            EBG = 14                         # eb blocks per bulk load
            for g in range(EB // EBG):
                ech = echpool.tile([P, EBG * P], dt.float16)
                sp_g = g * EBG // (ECHK * ECW // P)
                c0 = g * EBG * P - sp_g * ECHK * ECW
                nc.sync.dma_start(out=ech[:],
                                  in_=esum_red[sp_g, :, c0:c0 + EBG * P])
                for s in range(EBG):
                    eb = g * EBG + s
                    pst = psA.tile([P, P], dt.float16, space="PSUM",
                                   name=f"ps{s % 2}")
                    nc.tensor.transpose(pst[:], ech[:, s * P:(s + 1) * P],
                                        id_sb[:])
                    nc.vector.tensor_scalar_mul(out=ef_sb[:, eb, :], in0=pst[:],
                                                scalar1=re_sb[:, eb:eb + 1])

            # ---- phase B: out_cm[c, v] = sum_eb ef[eb]^T @ A2[eb, vchunk] ----
            for sp in range(VPASS):
                pss = [psA.tile([P, VCW], dt.float32, space="PSUM",
                                name=f"ps{j}") for j in range(VCHK)]
                for eb in range(EB):
                    a2t = a2pool.tile([P, VCHK * VCW], dt.float8e4)
                    r0 = (sp * EB + eb) * P
                    nc.sync.dma_start(out=a2t[:], in_=a2[r0:r0 + P, :])
                    for j in range(VCHK):
                        nc.tensor.matmul(out=pss[j][:],
                                         lhsT=ef_sb[:, eb, :],
                                         rhs=a2t[:, j * VCW:(j + 1) * VCW],
                                         start=(eb == 0), stop=(eb == EB - 1))
                for j in range(VCHK):
                    cm = work.tile([P, VCW], dt.float16)
                    nc.vector.tensor_copy(out=cm[:], in_=pss[j][:])
                    for b in range(VCW // P):
                        vbb = (sp * VCHK + j) * (VCW // P) + b
                        pst = psA.tile([P, P], dt.float16, space="PSUM",
                                       name=f"ps{5 + b % 2}")
                        nc.tensor.transpose(pst[:], cm[:, b * P:(b + 1) * P],
                                            id_sb[:])
                        ot = work.tile([P, OUT_CH], dt.float32)
                        nc.vector.tensor_scalar(out=ot[:], in0=pst[:],
                                                scalar1=rv_sb[:, vbb:vbb + 1],
                                                scalar2=0.0,
                                                op0=mybir.AluOpType.mult,
                                                op1=mybir.AluOpType.max)
                        nc.sync.dma_start(out=out[vbb * P:(vbb + 1) * P, :],
                                          in_=ot[:])

    nc.compile()
    return nc


def pack_inputs(X, W, b, v_idx, e_idx):
    """Host-side preprocessing: build per-core fp8 incidence tiles in the
    pass/block-chunk layouts the device program streams, plus dense inputs."""
    import ml_dtypes
    f16, f32 = np.float16, np.float32
    f8 = ml_dtypes.float8_e4m3

    v = np.asarray(v_idx).astype(np.int64)
    e = np.asarray(e_idx).astype(np.int64)

    # fp8 byte LUT for small counts (0..15); counts beyond 15 are impossible
    # for random data but clip defensively (value error stays tiny/local).
    lut = np.arange(16, dtype=np.float32).astype(f8).view(np.uint8)

    # dense inputs
    xt_full = np.zeros((IN_CH, NV_PAD), f16)
    xt_full[:, :N_VERTICES] = np.asarray(X, f32).T.astype(f16)
    wt = np.ascontiguousarray(np.asarray(W, f32).T.astype(f16))
    bmat = np.tile(np.asarray(b, f32)[None, :], (P, 1))
    ident = np.eye(P, dtype=f16)

    # degree reciprocals
    edeg = np.bincount(e, minlength=NE_PAD).astype(f32)
    re = (1.0 / np.maximum(edeg, 1.0)).astype(f32)
    re_p = np.ascontiguousarray(re.reshape(EB, P).T)          # [er, eb]
    vdeg = np.bincount(v, minlength=N_CORES * VPC_B).astype(f32)
    rv = (1.0 / np.maximum(vdeg, 1.0)).astype(f32)

    core = v // VPC
    vl = v - core * VPC

    def counts_to_f8(cnt_u8, rows, cols):
        # fp8 e4m3 byte for 1.0 is 0x38; counts are overwhelmingly 0/1, so a
        # byte-multiply covers them and the rare multi-edges get LUT-fixed.
        out = cnt_u8 * np.uint8(0x38)
        fix = np.flatnonzero(cnt_u8 > 1)
        if fix.size:
            out[fix] = lut[np.minimum(cnt_u8[fix], 15)].copy()
        return out.view(f8).reshape(rows, cols)

    in_maps = []
    for c in range(N_CORES):
        m = core == c
        vc, ec = vl[m], e[m]

        # a1[(sp*VB+vb)*P + vr, ecp*ECW + el] = count(v==vb*P+vr,
        #                                             e==(sp*ECHK+ecp)*ECW+el)
        EC = ECHK * ECW
        row1 = (ec // EC * VB + vc // P) * P + vc % P
        cnt = np.zeros(EPASS * VB * P * EC, np.uint8)
        np.add.at(cnt, row1 * EC + ec % EC, 1)
        a1 = counts_to_f8(cnt, EPASS * VB * P, EC)
        del cnt

        # a2[(sp*EB+eb)*P + er, vcp*VCW + vl] = count(e==eb*P+er,
        #                                             v==(sp*VCHK+vcp)*VCW+vl)
        VC = VCHK * VCW
        row2 = (vc // VC * EB + ec // P) * P + ec % P
        cnt = np.zeros(VPASS * EB * P * VC, np.uint8)
        np.add.at(cnt, row2 * VC + vc % VC, 1)
        a2 = counts_to_f8(cnt, VPASS * EB * P, VC)
        del cnt

        rv_core = rv[c * VPC:(c + 1) * VPC]
        rv_pad = np.zeros(VPC_B, f32)
        rv_pad[:VPC] = rv_core
        in_maps.append({
            "xt": np.ascontiguousarray(xt_full[:, c * VPC:(c + 1) * VPC]),
            "wt": wt,
            "bmat": bmat,
            "ident": ident,
            "a1": a1,
            "a2": a2,
            "re": re_p,
            "rv": np.ascontiguousarray(rv_pad.reshape(VBB, P).T),
        })
    return in_maps


def run(in_maps, trace=False):
    global LAST_RESULTS
    from concourse.bass_utils import run_bass_kernel_spmd
    if "prog" not in _PROG_CACHE:
        _PROG_CACHE["prog"] = build_program()
    nc = _PROG_CACHE["prog"]
    res = run_bass_kernel_spmd(nc, in_maps, core_ids=list(range(N_CORES)),
                               trace=trace)
    LAST_RESULTS = res
    return res


def kernel(X, W, b, v_idx, e_idx, trace=False):
    global LAST_IN_MAPS
    in_maps = pack_inputs(X, W, b, v_idx, e_idx)
    LAST_IN_MAPS = in_maps
    res = run(in_maps, trace=trace)
    out = np.concatenate([res.results[c]["out"][:VPC] for c in range(N_CORES)],
                         axis=0)
    return np.ascontiguousarray(out[:N_VERTICES]).astype(np.float32)
